# revision 23
# baseline (speedup 1.0000x reference)
"""Trainium2 Bass kernel for CompressedLinear (VQ codebook linear layer).

Computes: out = x @ W^T + bias, where
  W = (centroids[indices] @ Pi) * row_norms[:, None]

Sharding: out_features (4096) split across 8 cores (512 each); x replicated.
Per-core device pipeline:
  1. Gather yts[j,o] = centroids[idxT[j,o]] via fused custom-DVE ops
     (2 codebook entries per instruction, 8 instructions per chain). Chains
     run on wide tiles (CHAIN_SIZES j-blocks packed along the free dim) to
     amortize per-op overhead; the serial DVE chain is the kernel's
     critical path for the first ~150us, so the first chain is tiny (fast
     DMA, early start) and the last is a single j-block (minimal spill of
     stage-2 pass 0 past the gather).
  2. W_u^T[i,o] = sum_j Pi[j,i] * yts[j,o] on the PE (bf16, f32 psum),
     j-major over 8 psum banks x 4 passes; pass 0 overlaps the gather
     (psum capacity caps this overlap at 1/4 of stage 2). Drains alternate
     Act/DVE so bank reuse isn't gated on one queue.
  3. outT[o,t] = sum_i W_u^T[i,o] * xT[i,t] over 16 token groups of 512,
     4 psum chains per group alternating bank sets between groups (no
     bank-turnaround stall); out = rn*acc + bias alternates DVE and Act
     (activation Identity with per-partition scale/bias APs); xr DMAs
     alternate sync/gpsimd issue queues with deep prefetch to ride out
     HBM jitter.
Host feeds x pre-transposed/bf16-cast (layout prep), Pi in column-stripe
layout, indices transposed+packed; host reassembles the 8 outT shards.
"""

import numpy as np

# Problem geometry (hardcoded per contract)
OUT, IN = 4096, 4096
B, S = 4, 2048
T = B * S          # 8192 tokens
NCORES = 8
P = 128            # partitions

_DVE_OPS = None
_NC_CACHE = {}


def _register_dve_ops():
    """Register the fused VQ-gather ops in dve_ops.OPS (idempotent).

    VQ_PAIR covers codebook entries {imm2, imm2+1}; VQ_ACC2 accumulates two
    more on top of Src1. 8 instructions cover all 16 entries. All bf16
    (exact: per element exactly one eq-term is nonzero, so every partial
    sum is 0 or bf16(c_k) — no rounding drift).
    """
    global _DVE_OPS
    if _DVE_OPS is not None:
        return _DVE_OPS
    import concourse.dve_ops as dvo
    from concourse.dve_spec import (
        Spec, Src0, Src1, C0, C1, C2, One, eq, lower,
    )
    from concourse.dve_uop import DveOpSpec

    existing = {op.name: op for op in dvo.OPS}
    if "VQ_PAIR" in existing:
        _DVE_OPS = {k: existing[k] for k in ("VQ_PAIR", "VQ_ACC2")}
        return _DVE_OPS

    ver = "v3"  # TRN2

    def mk(name, spec, rd1):
        opcode = dvo._CUSTOM_DVE_ROW_BASE + len(dvo.OPS)
        dvo._SUB_OPCODE_FOR_NAME[name] = opcode
        s = DveOpSpec(name=name, opcode=opcode, uops=lower(spec, ver=ver), rd1_en=rd1)
        op = dvo.DveOp(name, spec, subdim=False, uops_sha={ver: s.sha(ver)})
        dvo.OPS.append(op)
        dvo.CUSTOM_DVE_SPECS[name] = spec
        return op

    # out = (idx==imm2)*s0 + (idx==imm2+1)*s1
    pair = mk(
        "VQ_PAIR",
        Spec(
            body=eq(Src0, C2) * C0 + eq(Src0, C2 + One) * C1,
            reference=lambda in0, in1, s0, s1, imm2: (
                (in0 == imm2) * s0 + (in0 == imm2 + 1) * s1
            ).astype(np.float32),
        ),
        False,
    )
    # out = acc + (idx==imm2)*s0 + (idx==imm2+1)*s1
    acc = mk(
        "VQ_ACC2",
        Spec(
            body=Src1 + eq(Src0, C2) * C0 + eq(Src0, C2 + One) * C1,
            reference=lambda in0, in1, s0, s1, imm2: (
                in1 + (in0 == imm2) * s0 + (in0 == imm2 + 1) * s1
            ).astype(np.float32),
        ),
        True,
    )
    _DVE_OPS = {"VQ_PAIR": pair, "VQ_ACC2": acc}
    return _DVE_OPS


# Gather chain sizes (j-blocks per wide DVE chain) = stage-2 j-slices.
# Sized so cumulative gather production stays just ahead of the PE's sweep
# consumption (fine early for a fast start, coarse late): with these sizes
# the PE goes continuously busy from ~17us and never waits on the gather.
CHAIN_SIZES = (2, 3, 5, 9, 13)


def build_nc(cvals, in_=IN, t=T, osh=OUT // NCORES, tch=512):
    """Build the SPMD Bass program. cvals: 16 python floats (codebook)."""
    import concourse.bacc as bacc
    import concourse.mybir as mybir
    from concourse.tile import TileContext

    f32 = mybir.dt.float32
    bf16 = mybir.dt.bfloat16

    nj = in_ // P          # j blocks (rows of Pi / x input dim)
    ni = in_ // P          # i blocks (cols of Pi / contraction of main mm)
    nob = osh // P         # output feature blocks per core (4)
    nsub = 8               # stage-2 sub-passes per sweep (4 i-blocks each)
    nk = ni // nsub        # i-blocks per sub-pass (4)
    ngt = t // tch         # stage-3 token groups (16)

    nc = bacc.Bacc()
    xT_d = nc.dram_tensor("xT", [in_, t], bf16, kind="ExternalInput")
    piR_d = nc.dram_tensor("PiR", [nsub * nj, P, nk, P], bf16,
                           kind="ExternalInput")
    idxW_d = nc.dram_tensor("idxW", [P, nj * osh], bf16, kind="ExternalInput")
    eye_d = nc.dram_tensor("eye", [P, P], bf16, kind="ExternalInput")
    rn_d = nc.dram_tensor("rn", [osh], f32, kind="ExternalInput")
    bias_d = nc.dram_tensor("bias", [osh], f32, kind="ExternalInput")
    outT_d = nc.dram_tensor("outT", [osh, t], f32, kind="ExternalOutput")

    with TileContext(nc) as tc:
        with (
            tc.tile_pool(name="constp", bufs=1) as constp,
            tc.tile_pool(name="idxp", bufs=2) as idxp,
            tc.tile_pool(name="ytsp", bufs=1) as ytsp,
            tc.tile_pool(name="pip", bufs=6) as pip,
            tc.tile_pool(name="wtp", bufs=1) as wtp,
            tc.tile_pool(name="xrp", bufs=20) as xrp,
            tc.tile_pool(name="outp", bufs=4) as outp,
            tc.tile_pool(name="wpsum", bufs=1, space="PSUM") as wpsum,
        ):
            # ---- Stage 1: codebook gather: yts[j][p, o] = centroids[idxT] --
            # 4 wide chains of 8 fused custom-DVE ops on [P, 4096] tiles
            # (8 j-blocks packed along the free dim). Issue the first idx
            # DMA before anything else: the DVE chain is the critical path.
            ops = _register_dve_ops()
            vq_pair, vq_acc2 = ops["VQ_PAIR"], ops["VQ_ACC2"]
            ytsw = []       # (j_block_offset, width_in_blocks, tile)
            off = 0
            for m, cs in enumerate(CHAIN_SIZES):
                wf = cs * osh
                idx_t = idxp.tile([P, wf], bf16, name="idx_t", tag="idx",
                                  bufs=2)
                nc.sync.dma_start(
                    idx_t[:], idxW_d[:, off * osh:(off + cs) * osh])
                cur = idxp.tile([P, wf], bf16, name="g", tag="g", bufs=2)
                nc.vector._custom_dve(
                    vq_pair, out=cur[:], in0=idx_t[:],
                    s0=float(cvals[0]), s1=float(cvals[1]), imm2=0.0,
                )
                for k in range(2, 16, 2):
                    if k == 14:
                        dst = ytsp.tile([P, wf], bf16, name="y_t",
                                        tag=f"yts{m}")
                    else:
                        dst = idxp.tile([P, wf], bf16, name="g", tag="g",
                                        bufs=2)
                    nc.vector._custom_dve(
                        vq_acc2, out=dst[:], in0=idx_t[:], in1=cur[:],
                        s0=float(cvals[k]), s1=float(cvals[k + 1]),
                        imm2=float(k),
                    )
                    cur = dst
                ytsw.append((off, cs, cur))
                off += cs
            assert off == nj

            rn_sb = constp.tile([P, nob], f32, name="rn_sb")
            nc.scalar.dma_start(rn_sb[:], rn_d.rearrange("(b p) -> p b", p=P))
            bias_sb = constp.tile([P, nob], f32, name="bias_sb")
            nc.scalar.dma_start(bias_sb[:],
                                bias_d.rearrange("(b p) -> p b", p=P))
            eye_sb = constp.tile([P, P], bf16, name="eye_sb")
            nc.scalar.dma_start(eye_sb[:], eye_d[:, :])

            def yts_view(j):
                for o0, cs, tile in ytsw:
                    if o0 <= j < o0 + cs:
                        return tile[:, (j - o0) * osh:(j - o0 + 1) * osh]
                raise AssertionError(j)

            # ---- Stage 2: wt[i_blk][p_i, o] = sum_j Pi[j, i] * yts[j, o] ---
            # One sweep per gather chain: ALL 32 i-blocks consume that
            # chain's j-slice as soon as it lands, in 8 sub-passes of 4
            # psum chains on alternating bank sets (no bank-turnaround
            # stall). Each sub-pass closes its chains and spills the
            # partial W to SBUF (bf16); the next sweep re-injects it via
            # an identity matmul at the END of the chain (by which time
            # the spill has long completed). This breaks the psum-capacity
            # cap on gather/stage-2 overlap: the PE saturates from ~17us.
            wts = [None] * ni
            nsweep = len(CHAIN_SIZES)
            off = 0
            for kk, cs in enumerate(CHAIN_SIZES):
                js = range(off, off + cs)
                off += cs
                # Partial re-injection mode per sweep: early sweeps are
                # gather-gated, so a PE identity-matmul reload is free
                # (rides the PE's idle); late sweeps are PE-bound, so the
                # partial is instead WRITTEN into the freed bank by Act
                # (or DVE once the gather is done) and the chain
                # accumulates on top with start=False.
                use_init = kk >= 3
                dve_ok = kk == nsweep - 1   # starts after the gather ends
                if not use_init:
                    # Early sweeps: 8 sub-passes of 4 banks, partial
                    # re-injected by a PE identity-matmul at chain end.
                    for s in range(nsub):
                        bank = (s % 2) * nk
                        ps = [
                            wpsum.tile([P, osh], f32, name="wps",
                                       tag=f"wps{bank + c}")
                            for c in range(nk)
                        ]
                        for jj, j in enumerate(js):
                            pi_t = pip.tile([P, nk, P], bf16, name="pi_t",
                                            tag="pi")
                            nc.sync.dma_start(pi_t[:], piR_d[s * nj + j])
                            yv = yts_view(j)
                            for c in range(nk):
                                nc.tensor.matmul(
                                    ps[c][:], pi_t[:, c, :], yv,
                                    start=(jj == 0),
                                    stop=(jj == cs - 1 and kk == 0),
                                )
                        for c in range(nk):
                            i_blk = s * nk + c
                            if kk > 0:
                                nc.tensor.matmul(
                                    ps[c][:], eye_sb[:], wts[i_blk][:],
                                    start=False, stop=True,
                                )
                            wt_t = wtp.tile([P, osh], bf16, name="wt_t",
                                            tag=f"wt{i_blk}")
                            nc.scalar.copy(wt_t[:], ps[c][:])
                            wts[i_blk] = wt_t
                else:
                    # Late (PE-bound) sweeps: the partial is WRITTEN into
                    # the freed bank by Act/DVE and the chain accumulates
                    # on top (start=False). 16 sub-passes of 2 banks give
                    # rotation depth 4, so the init has ~3 sub-passes of
                    # slack and never stalls the PE. pi tiles are shared
                    # across sub-pass pairs (same DMA granularity).
                    eng = nc.vector if dve_ok else nc.scalar
                    pair_tiles = {}
                    for s2 in range(2 * nsub):
                        bank = (s2 % 4) * 2
                        half = s2 % 2
                        ps = [
                            wpsum.tile([P, osh], f32, name="wps",
                                       tag=f"wps{bank + c}")
                            for c in range(2)
                        ]
                        for c in range(2):
                            i_blk = s2 * 2 + c
                            if dve_ok:
                                nc.vector.tensor_copy(ps[c][:],
                                                      wts[i_blk][:])
                            else:
                                nc.scalar.copy(ps[c][:], wts[i_blk][:])
                        for jj, j in enumerate(js):
                            if half == 0:
                                pi_t = pip.tile([P, nk, P], bf16,
                                                name="pi_t", tag="pi")
                                nc.sync.dma_start(
                                    pi_t[:], piR_d[(s2 // 2) * nj + j])
                                pair_tiles[j] = pi_t
                            else:
                                pi_t = pair_tiles[j]
                            yv = yts_view(j)
                            for c in range(2):
                                nc.tensor.matmul(
                                    ps[c][:], pi_t[:, half * 2 + c, :], yv,
                                    start=False, stop=(jj == cs - 1),
                                )
                        for c in range(2):
                            i_blk = s2 * 2 + c
                            wt_t = wtp.tile([P, osh], bf16, name="wt_t",
                                            tag=f"wt{i_blk}")
                            if dve_ok:
                                nc.vector.tensor_copy(wt_t[:], ps[c][:])
                            else:
                                nc.scalar.copy(wt_t[:], ps[c][:])
                            wts[i_blk] = wt_t

            # ---- Stage 3: outT[o, t] = rn[o] * sum_i wt[i,o]*xT[i,t] + b[o]
            # 16 groups of 512 tokens; per group 4 psum chains (one per
            # 128-row output block), alternating bank sets between
            # consecutive groups so group g+1's matmuls never wait on
            # group g's drain. Drain work (scale+bias) alternates between
            # DVE and GpSimd; xr DMAs alternate sync/scalar queues.
            for g in range(ngt):
                bank = (g % 2) * nob
                mps = [
                    wpsum.tile([P, tch], f32, name="mp",
                               tag=f"wps{bank + ob}")
                    for ob in range(nob)
                ]
                for i_blk in range(ni):
                    xr = xrp.tile([P, tch], bf16, name="xr", tag="xr")
                    eng = nc.sync if (i_blk % 2 == 0) else nc.gpsimd
                    eng.dma_start(
                        xr[:],
                        xT_d[i_blk * P:(i_blk + 1) * P,
                             g * tch:(g + 1) * tch],
                    )
                    for ob in range(nob):
                        nc.tensor.matmul(
                            mps[ob][:], wts[i_blk][:, ob * P:(ob + 1) * P],
                            xr[:],
                            start=(i_blk == 0), stop=(i_blk == ni - 1),
                        )
                for ob in range(nob):
                    o_t = outp.tile([P, tch], f32, name="o_t", tag="out")
                    if ob % 2 == 0:
                        nc.vector.tensor_scalar(
                            o_t[:], mps[ob][:], rn_sb[:, ob:ob + 1],
                            bias_sb[:, ob:ob + 1],
                            mybir.AluOpType.mult, mybir.AluOpType.add,
                        )
                    else:
                        # out = Identity(in*scale + bias) on the Act engine
                        nc.scalar.activation(
                            o_t[:], mps[ob][:],
                            mybir.ActivationFunctionType.Identity,
                            bias=bias_sb[:, ob:ob + 1],
                            scale=rn_sb[:, ob:ob + 1],
                        )
                    nc.scalar.dma_start(
                        outT_d[ob * P:(ob + 1) * P,
                               g * tch:(g + 1) * tch],
                        o_t[:],
                    )
    nc.compile()
    return nc


def _prep_inputs(x, indices, Pi, row_norms, bias):
    """Host-side layout prep + sharding. Returns list of per-core in_maps."""
    import ml_dtypes

    bf16 = ml_dtypes.bfloat16
    x2 = np.ascontiguousarray(
        np.asarray(x, np.float32).reshape(T, IN).T
    ).astype(bf16)  # (IN, T)
    nj = IN // P
    nsub, nk = 8, (IN // P) // 8
    # piR[s*nj + j, jp, c, ii] = Pi[j*P + jp, (s*nk + c)*P + ii]
    piR = np.ascontiguousarray(
        np.asarray(Pi, np.float32).astype(bf16)
        .reshape(nj, P, nsub, nk, P).transpose(2, 0, 1, 3, 4)
        .reshape(nsub * nj, P, nk, P)
    )
    eye = np.eye(P, dtype=np.float32).astype(bf16)
    idxT = np.ascontiguousarray(np.asarray(indices).T).astype(bf16)  # (IN, OUT)
    rn = np.asarray(row_norms, np.float32)
    bs = np.asarray(bias, np.float32)

    osh = OUT // NCORES
    in_maps = []
    for c in range(NCORES):
        sl = slice(c * osh, (c + 1) * osh)
        # idxW[p, j_blk*osh + o] = idxT[j_blk*P + p, c*osh + o]
        idxW = np.ascontiguousarray(
            idxT[:, sl].reshape(nj, P, osh).transpose(1, 0, 2)
            .reshape(P, nj * osh)
        )
        in_maps.append({
            "xT": x2,
            "PiR": piR,
            "idxW": idxW,
            "eye": eye,
            "rn": np.ascontiguousarray(rn[sl]),
            "bias": np.ascontiguousarray(bs[sl]),
        })
    return in_maps


def _get_nc(centroids):
    key = np.asarray(centroids, np.float32).tobytes()
    nc = _NC_CACHE.get(key)
    if nc is None:
        cvals = [float(v) for v in np.asarray(centroids, np.float32)]
        assert len(cvals) == 16
        nc = build_nc(cvals)
        _NC_CACHE.clear()
        _NC_CACHE[key] = nc
    return nc


def kernel(x, indices, centroids, Pi, row_norms, bias):
    from concourse.bass_utils import run_bass_kernel_spmd

    nc = _get_nc(centroids)
    in_maps = _prep_inputs(x, indices, Pi, row_norms, bias)
    res = run_bass_kernel_spmd(nc, in_maps, list(range(NCORES)))
    shards = [np.asarray(res.results[c]["outT"]) for c in range(NCORES)]
    full = np.concatenate(shards, axis=0)           # (OUT, T)
    out = np.ascontiguousarray(full.T).reshape(B, S, OUT)
    return out.astype(np.float32)


# revision 24
# speedup vs baseline: 1.0774x; 1.0774x over previous
"""Trainium2 Bass kernel for CompressedLinear (VQ codebook linear layer).

Computes: out = x @ W^T + bias, where
  W = (centroids[indices] @ Pi) * row_norms[:, None]

Sharding: out_features (4096) split across 8 cores (512 each); x replicated.
Per-core device pipeline:
  1. Gather yts[j,o] = centroids[idxT[j,o]] via fused custom-DVE ops
     (2 codebook entries per instruction, 8 instructions per chain). Chains
     run on wide tiles (CHAIN_SIZES j-blocks packed along the free dim) to
     amortize per-op overhead; the serial DVE chain is the kernel's
     critical path for the first ~150us, so the first chain is tiny (fast
     DMA, early start) and the last is a single j-block (minimal spill of
     stage-2 pass 0 past the gather).
  2. W_u^T[i,o] = sum_j Pi[j,i] * yts[j,o] on the PE (bf16, f32 psum),
     j-major over 8 psum banks x 4 passes; pass 0 overlaps the gather
     (psum capacity caps this overlap at 1/4 of stage 2). Drains alternate
     Act/DVE so bank reuse isn't gated on one queue.
  3. outT[o,t] = sum_i W_u^T[i,o] * xT[i,t] over 16 token groups of 512,
     4 psum chains per group alternating bank sets between groups (no
     bank-turnaround stall); out = rn*acc + bias alternates DVE and Act
     (activation Identity with per-partition scale/bias APs); xr DMAs
     alternate sync/gpsimd issue queues with deep prefetch to ride out
     HBM jitter.
Host feeds x pre-transposed/bf16-cast (layout prep), Pi in column-stripe
layout, indices transposed+packed; host reassembles the 8 outT shards.
"""

import numpy as np

# Problem geometry (hardcoded per contract)
OUT, IN = 4096, 4096
B, S = 4, 2048
T = B * S          # 8192 tokens
NCORES = 8
P = 128            # partitions

_DVE_OPS = None
_NC_CACHE = {}


def _register_dve_ops():
    """Register the fused VQ-gather ops in dve_ops.OPS (idempotent).

    VQ_PAIR covers codebook entries {imm2, imm2+1}; VQ_ACC2 accumulates two
    more on top of Src1. 8 instructions cover all 16 entries. All bf16
    (exact: per element exactly one eq-term is nonzero, so every partial
    sum is 0 or bf16(c_k) — no rounding drift).
    """
    global _DVE_OPS
    if _DVE_OPS is not None:
        return _DVE_OPS
    import concourse.dve_ops as dvo
    from concourse.dve_spec import (
        Spec, Src0, Src1, C0, C1, C2, One, eq, lower,
    )
    from concourse.dve_uop import DveOpSpec

    existing = {op.name: op for op in dvo.OPS}
    if "VQ_PAIR" in existing:
        _DVE_OPS = {k: existing[k] for k in ("VQ_PAIR", "VQ_ACC2")}
        return _DVE_OPS

    ver = "v3"  # TRN2

    def mk(name, spec, rd1):
        opcode = dvo._CUSTOM_DVE_ROW_BASE + len(dvo.OPS)
        dvo._SUB_OPCODE_FOR_NAME[name] = opcode
        s = DveOpSpec(name=name, opcode=opcode, uops=lower(spec, ver=ver), rd1_en=rd1)
        op = dvo.DveOp(name, spec, subdim=False, uops_sha={ver: s.sha(ver)})
        dvo.OPS.append(op)
        dvo.CUSTOM_DVE_SPECS[name] = spec
        return op

    # out = (idx==imm2)*s0 + (idx==imm2+1)*s1
    pair = mk(
        "VQ_PAIR",
        Spec(
            body=eq(Src0, C2) * C0 + eq(Src0, C2 + One) * C1,
            reference=lambda in0, in1, s0, s1, imm2: (
                (in0 == imm2) * s0 + (in0 == imm2 + 1) * s1
            ).astype(np.float32),
        ),
        False,
    )
    # out = acc + (idx==imm2)*s0 + (idx==imm2+1)*s1
    acc = mk(
        "VQ_ACC2",
        Spec(
            body=Src1 + eq(Src0, C2) * C0 + eq(Src0, C2 + One) * C1,
            reference=lambda in0, in1, s0, s1, imm2: (
                in1 + (in0 == imm2) * s0 + (in0 == imm2 + 1) * s1
            ).astype(np.float32),
        ),
        True,
    )
    _DVE_OPS = {"VQ_PAIR": pair, "VQ_ACC2": acc}
    return _DVE_OPS


# Gather chain sizes (j-blocks per wide DVE chain) = stage-2 j-slices.
# Sized so cumulative gather production stays just ahead of the PE's sweep
# consumption (fine early for a fast start, coarse late): with these sizes
# the PE goes continuously busy from ~17us and never waits on the gather.
CHAIN_SIZES = (2, 3, 5, 9, 13)


def build_nc(cvals, in_=IN, t=T, osh=OUT // NCORES, tch=512):
    """Build the SPMD Bass program. cvals: 16 python floats (codebook)."""
    import concourse.bacc as bacc
    import concourse.mybir as mybir
    from concourse.tile import TileContext

    f32 = mybir.dt.float32
    bf16 = mybir.dt.bfloat16

    nj = in_ // P          # j blocks (rows of Pi / x input dim)
    ni = in_ // P          # i blocks (cols of Pi / contraction of main mm)
    nob = osh // P         # output feature blocks per core (4)
    nsub = 8               # stage-2 sub-passes per sweep (4 i-blocks each)
    nk = ni // nsub        # i-blocks per sub-pass (4)
    ngt = t // tch         # stage-3 token groups (16)

    nc = bacc.Bacc()
    xT_d = nc.dram_tensor("xT", [in_, t], bf16, kind="ExternalInput")
    piR_d = nc.dram_tensor("PiR", [nsub * nj, P, nk, P], bf16,
                           kind="ExternalInput")
    idxW_d = nc.dram_tensor("idxW", [P, nj * osh], bf16, kind="ExternalInput")
    eye_d = nc.dram_tensor("eye", [P, P], bf16, kind="ExternalInput")
    rn_d = nc.dram_tensor("rn", [osh], f32, kind="ExternalInput")
    bias_d = nc.dram_tensor("bias", [osh], f32, kind="ExternalInput")
    outT_d = nc.dram_tensor("outT", [osh, t], f32, kind="ExternalOutput")

    with TileContext(nc) as tc:
        with (
            tc.tile_pool(name="constp", bufs=1) as constp,
            tc.tile_pool(name="idxp", bufs=2) as idxp,
            tc.tile_pool(name="ytsp", bufs=1) as ytsp,
            tc.tile_pool(name="pip", bufs=6) as pip,
            tc.tile_pool(name="wtp", bufs=1) as wtp,
            tc.tile_pool(name="xrp", bufs=20) as xrp,
            tc.tile_pool(name="outp", bufs=4) as outp,
            tc.tile_pool(name="wpsum", bufs=1, space="PSUM") as wpsum,
        ):
            # ---- Stage 1: codebook gather: yts[j][p, o] = centroids[idxT] --
            # 4 wide chains of 8 fused custom-DVE ops on [P, 4096] tiles
            # (8 j-blocks packed along the free dim). Issue the first idx
            # DMA before anything else: the DVE chain is the critical path.
            ops = _register_dve_ops()
            vq_pair, vq_acc2 = ops["VQ_PAIR"], ops["VQ_ACC2"]
            ytsw = []       # (j_block_offset, width_in_blocks, tile)
            off = 0
            for m, cs in enumerate(CHAIN_SIZES):
                wf = cs * osh
                idx_t = idxp.tile([P, wf], bf16, name="idx_t", tag="idx",
                                  bufs=2)
                nc.sync.dma_start(
                    idx_t[:], idxW_d[:, off * osh:(off + cs) * osh])
                cur = idxp.tile([P, wf], bf16, name="g", tag="g", bufs=2)
                nc.vector._custom_dve(
                    vq_pair, out=cur[:], in0=idx_t[:],
                    s0=float(cvals[0]), s1=float(cvals[1]), imm2=0.0,
                )
                for k in range(2, 16, 2):
                    if k == 14:
                        dst = ytsp.tile([P, wf], bf16, name="y_t",
                                        tag=f"yts{m}")
                    else:
                        dst = idxp.tile([P, wf], bf16, name="g", tag="g",
                                        bufs=2)
                    nc.vector._custom_dve(
                        vq_acc2, out=dst[:], in0=idx_t[:], in1=cur[:],
                        s0=float(cvals[k]), s1=float(cvals[k + 1]),
                        imm2=float(k),
                    )
                    cur = dst
                ytsw.append((off, cs, cur))
                off += cs
            assert off == nj

            rn_sb = constp.tile([P, nob], f32, name="rn_sb")
            nc.scalar.dma_start(rn_sb[:], rn_d.rearrange("(b p) -> p b", p=P))
            bias_sb = constp.tile([P, nob], f32, name="bias_sb")
            nc.scalar.dma_start(bias_sb[:],
                                bias_d.rearrange("(b p) -> p b", p=P))
            eye_sb = constp.tile([P, P], bf16, name="eye_sb")
            nc.scalar.dma_start(eye_sb[:], eye_d[:, :])

            def yts_view(j):
                for o0, cs, tile in ytsw:
                    if o0 <= j < o0 + cs:
                        return tile[:, (j - o0) * osh:(j - o0 + 1) * osh]
                raise AssertionError(j)

            # ---- Stage 2: wt[i_blk][p_i, o] = sum_j Pi[j, i] * yts[j, o] ---
            # One sweep per gather chain: ALL 32 i-blocks consume that
            # chain's j-slice as soon as it lands, in 8 sub-passes of 4
            # psum chains on alternating bank sets (no bank-turnaround
            # stall). Each sub-pass closes its chains and spills the
            # partial W to SBUF (bf16); the next sweep re-injects it via
            # an identity matmul at the END of the chain (by which time
            # the spill has long completed). This breaks the psum-capacity
            # cap on gather/stage-2 overlap: the PE saturates from ~17us.
            wts = [None] * ni
            off = 0
            for kk, cs in enumerate(CHAIN_SIZES):
                js = range(off, off + cs)
                off += cs
                last = off == nj
                for s in range(nsub):
                    bank = (s % 2) * nk
                    ps = [
                        wpsum.tile([P, osh], f32, name="wps",
                                   tag=f"wps{bank + c}")
                        for c in range(nk)
                    ]
                    for jj, j in enumerate(js):
                        pi_t = pip.tile([P, nk, P], bf16, name="pi_t",
                                        tag="pi")
                        nc.sync.dma_start(pi_t[:], piR_d[s * nj + j])
                        yv = yts_view(j)
                        for c in range(nk):
                            nc.tensor.matmul(
                                ps[c][:], pi_t[:, c, :], yv,
                                start=(jj == 0),
                                stop=(kk == 0 and jj == cs - 1),
                            )
                    for c in range(nk):
                        i_blk = s * nk + c
                        if kk > 0:
                            nc.tensor.matmul(
                                ps[c][:], eye_sb[:], wts[i_blk][:],
                                start=False, stop=True,
                            )
                        wt_t = wtp.tile([P, osh], bf16, name="wt_t",
                                        tag=f"wt{i_blk}")
                        nc.scalar.copy(wt_t[:], ps[c][:])
                        wts[i_blk] = wt_t

            # ---- Stage 3: outT[o, t] = rn[o] * sum_i wt[i,o]*xT[i,t] + b[o]
            # 16 groups of 512 tokens; per group 4 psum chains (one per
            # 128-row output block), alternating bank sets between
            # consecutive groups so group g+1's matmuls never wait on
            # group g's drain. Drain work (scale+bias) alternates between
            # DVE and GpSimd; xr DMAs alternate sync/scalar queues.
            for g in range(ngt):
                bank = (g % 2) * nob
                mps = [
                    wpsum.tile([P, tch], f32, name="mp",
                               tag=f"wps{bank + ob}")
                    for ob in range(nob)
                ]
                for i_blk in range(ni):
                    xr = xrp.tile([P, tch], bf16, name="xr", tag="xr")
                    eng = nc.sync if (i_blk % 2 == 0) else nc.gpsimd
                    eng.dma_start(
                        xr[:],
                        xT_d[i_blk * P:(i_blk + 1) * P,
                             g * tch:(g + 1) * tch],
                    )
                    for ob in range(nob):
                        nc.tensor.matmul(
                            mps[ob][:], wts[i_blk][:, ob * P:(ob + 1) * P],
                            xr[:],
                            start=(i_blk == 0), stop=(i_blk == ni - 1),
                        )
                for ob in range(nob):
                    o_t = outp.tile([P, tch], f32, name="o_t", tag="out")
                    if ob % 2 == 0:
                        nc.vector.tensor_scalar(
                            o_t[:], mps[ob][:], rn_sb[:, ob:ob + 1],
                            bias_sb[:, ob:ob + 1],
                            mybir.AluOpType.mult, mybir.AluOpType.add,
                        )
                    else:
                        # out = Identity(in*scale + bias) on the Act engine
                        nc.scalar.activation(
                            o_t[:], mps[ob][:],
                            mybir.ActivationFunctionType.Identity,
                            bias=bias_sb[:, ob:ob + 1],
                            scale=rn_sb[:, ob:ob + 1],
                        )
                    nc.scalar.dma_start(
                        outT_d[ob * P:(ob + 1) * P,
                               g * tch:(g + 1) * tch],
                        o_t[:],
                    )
    nc.compile()
    return nc


def _prep_inputs(x, indices, Pi, row_norms, bias):
    """Host-side layout prep + sharding. Returns list of per-core in_maps."""
    import ml_dtypes

    bf16 = ml_dtypes.bfloat16
    x2 = np.ascontiguousarray(
        np.asarray(x, np.float32).reshape(T, IN).T
    ).astype(bf16)  # (IN, T)
    nj = IN // P
    nsub, nk = 8, (IN // P) // 8
    # piR[s*nj + j, jp, c, ii] = Pi[j*P + jp, (s*nk + c)*P + ii]
    piR = np.ascontiguousarray(
        np.asarray(Pi, np.float32).astype(bf16)
        .reshape(nj, P, nsub, nk, P).transpose(2, 0, 1, 3, 4)
        .reshape(nsub * nj, P, nk, P)
    )
    eye = np.eye(P, dtype=np.float32).astype(bf16)
    idxT = np.ascontiguousarray(np.asarray(indices).T).astype(bf16)  # (IN, OUT)
    rn = np.asarray(row_norms, np.float32)
    bs = np.asarray(bias, np.float32)

    osh = OUT // NCORES
    in_maps = []
    for c in range(NCORES):
        sl = slice(c * osh, (c + 1) * osh)
        # idxW[p, j_blk*osh + o] = idxT[j_blk*P + p, c*osh + o]
        idxW = np.ascontiguousarray(
            idxT[:, sl].reshape(nj, P, osh).transpose(1, 0, 2)
            .reshape(P, nj * osh)
        )
        in_maps.append({
            "xT": x2,
            "PiR": piR,
            "idxW": idxW,
            "eye": eye,
            "rn": np.ascontiguousarray(rn[sl]),
            "bias": np.ascontiguousarray(bs[sl]),
        })
    return in_maps


def _get_nc(centroids):
    key = np.asarray(centroids, np.float32).tobytes()
    nc = _NC_CACHE.get(key)
    if nc is None:
        cvals = [float(v) for v in np.asarray(centroids, np.float32)]
        assert len(cvals) == 16
        nc = build_nc(cvals)
        _NC_CACHE.clear()
        _NC_CACHE[key] = nc
    return nc


def kernel(x, indices, centroids, Pi, row_norms, bias):
    from concourse.bass_utils import run_bass_kernel_spmd

    nc = _get_nc(centroids)
    in_maps = _prep_inputs(x, indices, Pi, row_norms, bias)
    res = run_bass_kernel_spmd(nc, in_maps, list(range(NCORES)))
    shards = [np.asarray(res.results[c]["outT"]) for c in range(NCORES)]
    full = np.concatenate(shards, axis=0)           # (OUT, T)
    out = np.ascontiguousarray(full.T).reshape(B, S, OUT)
    return out.astype(np.float32)


# revision 26
# speedup vs baseline: 1.0964x; 1.0176x over previous
"""Trainium2 Bass kernel for CompressedLinear (VQ codebook linear layer).

Computes: out = x @ W^T + bias, where
  W = (centroids[indices] @ Pi) * row_norms[:, None]

Sharding: out_features (4096) split across 8 cores (512 each); x replicated.
Per-core device pipeline:
  1. Gather yts[j,o] = centroids[idxT[j,o]] via fused custom-DVE ops
     (2 codebook entries per instruction, 8 instructions per chain). Chains
     run on wide tiles (CHAIN_SIZES j-blocks packed along the free dim) to
     amortize per-op overhead; the serial DVE chain is the kernel's
     critical path for the first ~150us, so the first chain is tiny (fast
     DMA, early start) and the last is a single j-block (minimal spill of
     stage-2 pass 0 past the gather).
  2. W_u^T[i,o] = sum_j Pi[j,i] * yts[j,o] on the PE (bf16, f32 psum),
     j-major over 8 psum banks x 4 passes; pass 0 overlaps the gather
     (psum capacity caps this overlap at 1/4 of stage 2). Drains alternate
     Act/DVE so bank reuse isn't gated on one queue.
  3. outT[o,t] = sum_i W_u^T[i,o] * xT[i,t] over 16 token groups of 512,
     4 psum chains per group alternating bank sets between groups (no
     bank-turnaround stall); out = rn*acc + bias alternates DVE and Act
     (activation Identity with per-partition scale/bias APs); xr DMAs
     alternate sync/gpsimd issue queues with deep prefetch to ride out
     HBM jitter.
Host feeds x pre-transposed/bf16-cast (layout prep), Pi in column-stripe
layout, indices transposed+packed; host reassembles the 8 outT shards.
"""

import numpy as np

# Problem geometry (hardcoded per contract)
OUT, IN = 4096, 4096
B, S = 4, 2048
T = B * S          # 8192 tokens
NCORES = 8
P = 128            # partitions

_DVE_OPS = None
_NC_CACHE = {}


def _register_dve_ops():
    """Register the fused VQ-gather ops in dve_ops.OPS (idempotent).

    VQ_PAIR covers codebook entries {imm2, imm2+1}; VQ_ACC2 accumulates two
    more on top of Src1. 8 instructions cover all 16 entries. All bf16
    (exact: per element exactly one eq-term is nonzero, so every partial
    sum is 0 or bf16(c_k) — no rounding drift).
    """
    global _DVE_OPS
    if _DVE_OPS is not None:
        return _DVE_OPS
    import concourse.dve_ops as dvo
    from concourse.dve_spec import (
        Spec, Src0, Src1, C0, C1, C2, One, eq, lower,
    )
    from concourse.dve_uop import DveOpSpec

    existing = {op.name: op for op in dvo.OPS}
    if "VQ_PAIR" in existing:
        _DVE_OPS = {k: existing[k] for k in ("VQ_PAIR", "VQ_ACC2")}
        return _DVE_OPS

    ver = "v3"  # TRN2

    def mk(name, spec, rd1):
        opcode = dvo._CUSTOM_DVE_ROW_BASE + len(dvo.OPS)
        dvo._SUB_OPCODE_FOR_NAME[name] = opcode
        s = DveOpSpec(name=name, opcode=opcode, uops=lower(spec, ver=ver), rd1_en=rd1)
        op = dvo.DveOp(name, spec, subdim=False, uops_sha={ver: s.sha(ver)})
        dvo.OPS.append(op)
        dvo.CUSTOM_DVE_SPECS[name] = spec
        return op

    # out = (idx==imm2)*s0 + (idx==imm2+1)*s1
    pair = mk(
        "VQ_PAIR",
        Spec(
            body=eq(Src0, C2) * C0 + eq(Src0, C2 + One) * C1,
            reference=lambda in0, in1, s0, s1, imm2: (
                (in0 == imm2) * s0 + (in0 == imm2 + 1) * s1
            ).astype(np.float32),
        ),
        False,
    )
    # out = acc + (idx==imm2)*s0 + (idx==imm2+1)*s1
    acc = mk(
        "VQ_ACC2",
        Spec(
            body=Src1 + eq(Src0, C2) * C0 + eq(Src0, C2 + One) * C1,
            reference=lambda in0, in1, s0, s1, imm2: (
                in1 + (in0 == imm2) * s0 + (in0 == imm2 + 1) * s1
            ).astype(np.float32),
        ),
        True,
    )
    _DVE_OPS = {"VQ_PAIR": pair, "VQ_ACC2": acc}
    return _DVE_OPS


# Gather chain sizes (j-blocks per wide DVE chain) = stage-2 j-slices.
# Sized so cumulative gather production stays just ahead of the PE's sweep
# consumption (fine early for a fast start, coarse late): with these sizes
# the PE goes continuously busy from ~17us and never waits on the gather.
CHAIN_SIZES = (2, 3, 5, 9, 13)


def build_nc(cvals, in_=IN, t=T, osh=OUT // NCORES, tch=512):
    """Build the SPMD Bass program. cvals: 16 python floats (codebook)."""
    import concourse.bacc as bacc
    import concourse.mybir as mybir
    from concourse.tile import TileContext

    f32 = mybir.dt.float32
    bf16 = mybir.dt.bfloat16

    nj = in_ // P          # j blocks (rows of Pi / x input dim)
    ni = in_ // P          # i blocks (cols of Pi / contraction of main mm)
    nob = osh // P         # output feature blocks per core (4)
    nsub = 8               # stage-2 sub-passes per sweep (4 i-blocks each)
    nk = ni // nsub        # i-blocks per sub-pass (4)
    ngt = t // tch         # stage-3 token groups (16)

    nc = bacc.Bacc()
    xT_d = nc.dram_tensor("xT", [in_, t], bf16, kind="ExternalInput")
    piR_d = nc.dram_tensor("PiR", [nsub * nj, P, nk, P], bf16,
                           kind="ExternalInput")
    idxW_d = nc.dram_tensor("idxW", [P, nj * osh], bf16, kind="ExternalInput")
    eye_d = nc.dram_tensor("eye", [P, P], bf16, kind="ExternalInput")
    rn_d = nc.dram_tensor("rn", [osh], f32, kind="ExternalInput")
    bias_d = nc.dram_tensor("bias", [osh], f32, kind="ExternalInput")
    outT_d = nc.dram_tensor("outT", [osh, t], f32, kind="ExternalOutput")

    with TileContext(nc) as tc:
        with (
            tc.tile_pool(name="constp", bufs=1) as constp,
            tc.tile_pool(name="idxp", bufs=2) as idxp,
            tc.tile_pool(name="ytsp", bufs=1) as ytsp,
            tc.tile_pool(name="pip", bufs=12) as pip,
            tc.tile_pool(name="wtp", bufs=1) as wtp,
            tc.tile_pool(name="xrp", bufs=24) as xrp,
            tc.tile_pool(name="outp", bufs=4) as outp,
            tc.tile_pool(name="wpsum", bufs=1, space="PSUM") as wpsum,
        ):
            # ---- Stage 1: codebook gather: yts[j][p, o] = centroids[idxT] --
            # 4 wide chains of 8 fused custom-DVE ops on [P, 4096] tiles
            # (8 j-blocks packed along the free dim). Issue the first idx
            # DMA before anything else: the DVE chain is the critical path.
            ops = _register_dve_ops()
            vq_pair, vq_acc2 = ops["VQ_PAIR"], ops["VQ_ACC2"]
            ytsw = []       # (j_block_offset, width_in_blocks, tile)
            off = 0
            for m, cs in enumerate(CHAIN_SIZES):
                wf = cs * osh
                idx_t = idxp.tile([P, wf], bf16, name="idx_t", tag="idx",
                                  bufs=2)
                nc.sync.dma_start(
                    idx_t[:], idxW_d[:, off * osh:(off + cs) * osh])
                cur = idxp.tile([P, wf], bf16, name="g", tag="g", bufs=2)
                nc.vector._custom_dve(
                    vq_pair, out=cur[:], in0=idx_t[:],
                    s0=float(cvals[0]), s1=float(cvals[1]), imm2=0.0,
                )
                for k in range(2, 16, 2):
                    if k == 14:
                        dst = ytsp.tile([P, wf], bf16, name="y_t",
                                        tag=f"yts{m}")
                    else:
                        dst = idxp.tile([P, wf], bf16, name="g", tag="g",
                                        bufs=2)
                    nc.vector._custom_dve(
                        vq_acc2, out=dst[:], in0=idx_t[:], in1=cur[:],
                        s0=float(cvals[k]), s1=float(cvals[k + 1]),
                        imm2=float(k),
                    )
                    cur = dst
                ytsw.append((off, cs, cur))
                off += cs
            assert off == nj

            rn_sb = constp.tile([P, nob], f32, name="rn_sb")
            nc.scalar.dma_start(rn_sb[:], rn_d.rearrange("(b p) -> p b", p=P))
            bias_sb = constp.tile([P, nob], f32, name="bias_sb")
            nc.scalar.dma_start(bias_sb[:],
                                bias_d.rearrange("(b p) -> p b", p=P))
            eye_sb = constp.tile([P, P], bf16, name="eye_sb")
            nc.scalar.dma_start(eye_sb[:], eye_d[:, :])

            def yts_view(j):
                for o0, cs, tile in ytsw:
                    if o0 <= j < o0 + cs:
                        return tile[:, (j - o0) * osh:(j - o0 + 1) * osh]
                raise AssertionError(j)

            # ---- Stage 2: wt[i_blk][p_i, o] = sum_j Pi[j, i] * yts[j, o] ---
            # One sweep per gather chain: ALL 32 i-blocks consume that
            # chain's j-slice as soon as it lands, in 8 sub-passes of 4
            # psum chains on alternating bank sets (no bank-turnaround
            # stall). Each sub-pass closes its chains and spills the
            # partial W to SBUF (bf16); the next sweep re-injects it via
            # an identity matmul at the END of the chain (by which time
            # the spill has long completed). This breaks the psum-capacity
            # cap on gather/stage-2 overlap: the PE saturates from ~17us.
            wts = [None] * ni
            off = 0
            for kk, cs in enumerate(CHAIN_SIZES):
                js = range(off, off + cs)
                off += cs
                last = off == nj
                for s in range(nsub):
                    bank = (s % 2) * nk
                    ps = [
                        wpsum.tile([P, osh], f32, name="wps",
                                   tag=f"wps{bank + c}")
                        for c in range(nk)
                    ]
                    for jj, j in enumerate(js):
                        pi_t = pip.tile([P, nk, P], bf16, name="pi_t",
                                        tag="pi")
                        nc.sync.dma_start(pi_t[:], piR_d[s * nj + j])
                        yv = yts_view(j)
                        for c in range(nk):
                            nc.tensor.matmul(
                                ps[c][:], pi_t[:, c, :], yv,
                                start=(jj == 0),
                                stop=(kk == 0 and jj == cs - 1),
                            )
                    for c in range(nk):
                        i_blk = s * nk + c
                        if kk > 0:
                            nc.tensor.matmul(
                                ps[c][:], eye_sb[:], wts[i_blk][:],
                                start=False, stop=True,
                            )
                        wt_t = wtp.tile([P, osh], bf16, name="wt_t",
                                        tag=f"wt{i_blk}")
                        nc.scalar.copy(wt_t[:], ps[c][:])
                        wts[i_blk] = wt_t

            # ---- Stage 3: outT[o, t] = rn[o] * sum_i wt[i,o]*xT[i,t] + b[o]
            # 16 groups of 512 tokens; per group 4 psum chains (one per
            # 128-row output block), alternating bank sets between
            # consecutive groups so group g+1's matmuls never wait on
            # group g's drain. Drain work (scale+bias) alternates between
            # DVE and GpSimd; xr DMAs alternate sync/scalar queues.
            for g in range(ngt):
                bank = (g % 2) * nob
                mps = [
                    wpsum.tile([P, tch], f32, name="mp",
                               tag=f"wps{bank + ob}")
                    for ob in range(nob)
                ]
                for i_blk in range(ni):
                    xr = xrp.tile([P, tch], bf16, name="xr", tag="xr")
                    eng = nc.sync if (i_blk % 2 == 0) else nc.gpsimd
                    eng.dma_start(
                        xr[:],
                        xT_d[i_blk * P:(i_blk + 1) * P,
                             g * tch:(g + 1) * tch],
                    )
                    for ob in range(nob):
                        nc.tensor.matmul(
                            mps[ob][:], wts[i_blk][:, ob * P:(ob + 1) * P],
                            xr[:],
                            start=(i_blk == 0), stop=(i_blk == ni - 1),
                        )
                for ob in range(nob):
                    o_t = outp.tile([P, tch], f32, name="o_t", tag="out")
                    if ob % 2 == 0:
                        nc.vector.tensor_scalar(
                            o_t[:], mps[ob][:], rn_sb[:, ob:ob + 1],
                            bias_sb[:, ob:ob + 1],
                            mybir.AluOpType.mult, mybir.AluOpType.add,
                        )
                    else:
                        # out = Identity(in*scale + bias) on the Act engine
                        nc.scalar.activation(
                            o_t[:], mps[ob][:],
                            mybir.ActivationFunctionType.Identity,
                            bias=bias_sb[:, ob:ob + 1],
                            scale=rn_sb[:, ob:ob + 1],
                        )
                    nc.scalar.dma_start(
                        outT_d[ob * P:(ob + 1) * P,
                               g * tch:(g + 1) * tch],
                        o_t[:],
                    )
    nc.compile()
    return nc


def _prep_inputs(x, indices, Pi, row_norms, bias):
    """Host-side layout prep + sharding. Returns list of per-core in_maps."""
    import ml_dtypes

    bf16 = ml_dtypes.bfloat16
    x2 = np.ascontiguousarray(
        np.asarray(x, np.float32).reshape(T, IN).T
    ).astype(bf16)  # (IN, T)
    nj = IN // P
    nsub, nk = 8, (IN // P) // 8
    # piR[s*nj + j, jp, c, ii] = Pi[j*P + jp, (s*nk + c)*P + ii]
    piR = np.ascontiguousarray(
        np.asarray(Pi, np.float32).astype(bf16)
        .reshape(nj, P, nsub, nk, P).transpose(2, 0, 1, 3, 4)
        .reshape(nsub * nj, P, nk, P)
    )
    eye = np.eye(P, dtype=np.float32).astype(bf16)
    idxT = np.ascontiguousarray(np.asarray(indices).T).astype(bf16)  # (IN, OUT)
    rn = np.asarray(row_norms, np.float32)
    bs = np.asarray(bias, np.float32)

    osh = OUT // NCORES
    in_maps = []
    for c in range(NCORES):
        sl = slice(c * osh, (c + 1) * osh)
        # idxW[p, j_blk*osh + o] = idxT[j_blk*P + p, c*osh + o]
        idxW = np.ascontiguousarray(
            idxT[:, sl].reshape(nj, P, osh).transpose(1, 0, 2)
            .reshape(P, nj * osh)
        )
        in_maps.append({
            "xT": x2,
            "PiR": piR,
            "idxW": idxW,
            "eye": eye,
            "rn": np.ascontiguousarray(rn[sl]),
            "bias": np.ascontiguousarray(bs[sl]),
        })
    return in_maps


def _get_nc(centroids):
    key = np.asarray(centroids, np.float32).tobytes()
    nc = _NC_CACHE.get(key)
    if nc is None:
        cvals = [float(v) for v in np.asarray(centroids, np.float32)]
        assert len(cvals) == 16
        nc = build_nc(cvals)
        _NC_CACHE.clear()
        _NC_CACHE[key] = nc
    return nc


def kernel(x, indices, centroids, Pi, row_norms, bias):
    from concourse.bass_utils import run_bass_kernel_spmd

    nc = _get_nc(centroids)
    in_maps = _prep_inputs(x, indices, Pi, row_norms, bias)
    res = run_bass_kernel_spmd(nc, in_maps, list(range(NCORES)))
    shards = [np.asarray(res.results[c]["outT"]) for c in range(NCORES)]
    full = np.concatenate(shards, axis=0)           # (OUT, T)
    out = np.ascontiguousarray(full.T).reshape(B, S, OUT)
    return out.astype(np.float32)
